# revision 25
# baseline (speedup 1.0000x reference)
"""Trainium2 Bass kernel for nn_Loop_Projection (batched per-prototype GEMM).

Computes out[b, e, p] = sum_d x[b, d, p] * W[p, d, e] + b[p, e] with
x: [256, 512, 128] f32, W: [128, 512, 128] f32, b: [128, 128] f32.

Sharding: prototype axis P=128 split across 8 NeuronCores (16 protos each).
Each core's x/W slices are pre-transposed on the host so every device DMA is
fully contiguous:
  xk[p][k, c*B + b] = x[b, 128c + k, p]      ([16, 128, 1024] per core)
  wk[p][k, c*E + e] = W[p, 128c + k, e]      ([16, 128, 512]  per core)
Per proto the kernel accumulates out.T = W_p.T @ x_p.T ([E, B] PSUM tile)
over 4 K-chunks of 128 (fp32 matmuls), adds the bias on the vector engine
during the PSUM->SBUF copy, and stores y[p] = [E, B] contiguous. The host
reassembles [B, E, P].

The device program is raw bacc (hand-placed semaphores, no Tile) so the
kernel has no Tile exit barrier. DMA traffic is spread over three rings --
x halves split across the two HWDGE rings (SP=sync + Act=scalar), W loads
alternating between them, stores on the gpsimd SWDGE ring -- which sustains
~400 GB/s aggregate vs ~260 GB/s for a single ring. Per-ring-slot DMA
semaphores are used because HWDGE completions of different DMAs can
interleave (only per-slot counts are race-free). Measured ~54-55 us on
8 cores (12 MiB in + 2 MiB out per core), rel err ~1e-7.
"""

import os

import numpy as np

import concourse.bass as bass
import concourse.tile as tile
from concourse import bacc, mybir
from concourse.bass_utils import run_bass_kernel_spmd

B, D, P, E = 256, 512, 128, 128
NCORES = 8
PL = P // NCORES  # prototypes per core
KC = D // 128  # contraction chunks of 128

_nc_cache = None
LAST_RESULTS = None  # BassKernelResults of the most recent run (for test.py)

USE_FP32R = False  # float32r matmul: 1 cycle/row vs 4 for float32
USE_RAW = True  # raw bacc (manual sems) instead of Tile: no ~9us exit barrier

NB = 8  # x/w sbuf ring depth
NPS = 8  # psum ring depth (8 banks)
NO = 16  # output slots: single-use, so no store-completion reuse guards


def _build_nc_raw() -> bass.Bass:
    nc = bacc.Bacc()
    xk = nc.dram_tensor("xk", [PL, 128, KC * B], mybir.dt.float32, kind="ExternalInput")
    wk = nc.dram_tensor("wk", [PL, 128, KC * E], mybir.dt.float32, kind="ExternalInput")
    bT = nc.dram_tensor("bT", [E, PL], mybir.dt.float32, kind="ExternalInput")
    y = nc.dram_tensor("y", [PL, E, B], mybir.dt.float32, kind="ExternalOutput")

    mm_dt = mybir.dt.float32r if USE_FP32R else mybir.dt.float32
    XW = KC * B  # 1024
    XH = XW // 2  # 512, per-ring half of an x tile

    # store issuer per proto: last two protos ride the HW rings (idle by then)
    def store_engine(p):
        if p == PL - 1:
            return "split"
        if p == PL - 2:
            return "act"
        return "pool"


    if True:
        # plain allocs (no context managers): freeing sems/tensors at the end
        # of the program emits a ~7us per-semaphore clear storm at kernel exit
        xbuf = [
            nc.alloc_sbuf_tensor(f"xbuf{i}", [128, XW], mm_dt).ap()
            for i in range(NB)
        ]
        wbuf = [
            nc.alloc_sbuf_tensor(f"wbuf{i}", [128, KC * E], mm_dt).ap()
            for i in range(NB)
        ]
        obuf = [
            nc.alloc_sbuf_tensor(f"obuf{i}", [E, B], mybir.dt.float32).ap()
            for i in range(NO)
        ]
        pbuf = [
            nc.alloc_psum_tensor(f"pbuf{i}", [E, B], mybir.dt.float32).ap()
            for i in range(NPS)
        ]
        btile = nc.alloc_sbuf_tensor("btile", [E, PL], mybir.dt.float32).ap()
        # one DMA-completion sem per ring slot: same-slot uses are serialized
        # by the ring guard, so per-slot counting is sound even though HWDGE
        # completions of different DMAs can interleave
        s_x = [nc.alloc_semaphore(f"s_x{i}") for i in range(NB)]
        s_st = nc.alloc_semaphore("s_st")
        s_st_hw = nc.alloc_semaphore("s_st_hw")
        s_b = nc.alloc_semaphore("s_b")
        s_mm = nc.alloc_semaphore("s_mm")
        s_vec = nc.alloc_semaphore("s_vec")

        with nc.Block() as block:

            @block.sync
            def _(sync: bass.BassEngine):
                for p in range(PL):
                    if p >= NB:
                        sync.wait_ge(s_mm, p - NB + 1)
                    sync.dma_start(
                        xbuf[p % NB][:, :XH], xk[p, :, :XH].bitcast(mm_dt)
                    ).then_inc(s_x[p % NB], 16)
                    if p % 2 == 0:
                        sync.dma_start(
                            wbuf[p % NB][:], wk[p].bitcast(mm_dt)
                        ).then_inc(s_x[p % NB], 16)
                p = PL - 2
                sync.wait_ge(s_vec, PL - 1)
                sync.dma_start(
                    y[p, :, : B // 2], obuf[p % NO][:, : B // 2]
                ).then_inc(s_st_hw, 16)
                p = PL - 1
                sync.wait_ge(s_vec, PL)
                sync.dma_start(
                    y[p, :, : B // 2], obuf[p % NO][:, : B // 2]
                ).then_inc(s_st_hw, 16)
                sync.wait_ge(s_st_hw, 64)

            @block.scalar
            def _(scalar: bass.BassEngine):
                scalar.dma_start(btile[:], bT[:]).then_inc(s_b, 16)
                for p in range(PL):
                    if p >= NB:
                        scalar.wait_ge(s_mm, p - NB + 1)
                    scalar.dma_start(
                        xbuf[p % NB][:, XH:], xk[p, :, XH:].bitcast(mm_dt)
                    ).then_inc(s_x[p % NB], 16)
                    if p % 2 == 1:
                        scalar.dma_start(
                            wbuf[p % NB][:], wk[p].bitcast(mm_dt)
                        ).then_inc(s_x[p % NB], 16)
                p = PL - 2
                scalar.wait_ge(s_vec, p + 1)
                scalar.dma_start(
                    y[p, :, B // 2 :], obuf[p % NO][:, B // 2 :]
                ).then_inc(s_st_hw, 16)
                p = PL - 1
                scalar.wait_ge(s_vec, p + 1)
                scalar.dma_start(
                    y[p, :, B // 2 :], obuf[p % NO][:, B // 2 :]
                ).then_inc(s_st_hw, 16)
                scalar.wait_ge(s_st_hw, 64)

            @block.tensor
            def _(tensor: bass.BassEngine):
                for p in range(PL):
                    i = p % NB
                    use = p // NB + 1
                    # both x halves + this slot's W load (3 DMAs x 16)
                    tensor.wait_ge(s_x[i], 48 * use)
                    if p >= NPS:
                        tensor.wait_ge(s_vec, p - NPS + 1)
                    for c in range(KC):
                        mm = nc.tensor.matmul(
                            pbuf[p % NPS][:],
                            lhsT=wbuf[i][:, c * E : (c + 1) * E],
                            rhs=xbuf[i][:, c * B : (c + 1) * B],
                            start=(c == 0),
                            stop=(c == KC - 1),
                        )
                    mm.then_inc(s_mm, 1)

            @block.vector
            def _(vector: bass.BassEngine):
                vector.wait_ge(s_b, 16)
                for p in range(PL):
                    vector.wait_ge(s_mm, p + 1)
                    nc.vector.tensor_scalar_add(
                        obuf[p % NO][:], pbuf[p % NPS][:], btile[:, p : p + 1]
                    ).then_inc(s_vec, 1)

            @block.gpsimd
            def _(gpsimd: bass.BassEngine):
                for p in range(PL):
                    if store_engine(p) != "pool":
                        continue
                    gpsimd.wait_ge(s_vec, p + 1)
                    gpsimd.dma_start(y[p], obuf[p % NO][:]).then_inc(s_st, 16)
                gpsimd.wait_ge(s_st, 16 * (PL - 2))

    nc.compile()
    return nc


def _build_nc() -> bass.Bass:
    if USE_RAW:
        return _build_nc_raw()
    nc = bacc.Bacc()
    xk = nc.dram_tensor("xk", [PL, 128, KC * B], mybir.dt.float32, kind="ExternalInput")
    wk = nc.dram_tensor("wk", [PL, 128, KC * E], mybir.dt.float32, kind="ExternalInput")
    bT = nc.dram_tensor("bT", [E, PL], mybir.dt.float32, kind="ExternalInput")
    y = nc.dram_tensor("y", [PL, E, B], mybir.dt.float32, kind="ExternalOutput")

    mm_dt = mybir.dt.float32r if USE_FP32R else mybir.dt.float32
    XW = KC * B  # 1024, x tile free width
    with tile.TileContext(nc) as tc:
        with (
            tc.tile_pool(name="const", bufs=1) as cpool,
            tc.tile_pool(name="xin", bufs=8) as xpool,
            tc.tile_pool(name="win", bufs=8) as wpool,
            tc.tile_pool(name="acc", bufs=8, space="PSUM") as ppool,
            tc.tile_pool(name="out", bufs=8) as opool,
        ):
            bt = cpool.tile([E, PL], mybir.dt.float32)
            nc.scalar.dma_start(bt[:], bT[:])
            for p in range(PL):
                # Split each x load across both HWDGE rings (SP + Act) and
                # alternate the W loads so both rings carry ~6 MiB; stores
                # ride the gpsimd SWDGE ring. One ring alone caps at ~260
                # GB/s, below the ~358 GB/s HBM-per-core limit.
                xt = xpool.tile([128, XW], mm_dt)
                nc.sync.dma_start(
                    xt[:, : XW // 2], xk[p, :, : XW // 2].bitcast(mm_dt)
                )
                nc.scalar.dma_start(
                    xt[:, XW // 2 :], xk[p, :, XW // 2 :].bitcast(mm_dt)
                )
                wt = wpool.tile([128, KC * E], mm_dt)
                weng = nc.sync if p % 2 == 0 else nc.scalar
                weng.dma_start(wt[:], wk[p].bitcast(mm_dt))
                ps = ppool.tile([E, B], mybir.dt.float32)
                for c in range(KC):
                    nc.tensor.matmul(
                        ps[:],
                        lhsT=wt[:, c * E : (c + 1) * E],
                        rhs=xt[:, c * B : (c + 1) * B],
                        start=(c == 0),
                        stop=(c == KC - 1),
                    )
                ot = opool.tile([E, B], mybir.dt.float32)
                # bias-add + PSUM->SBUF on the (otherwise idle) vector engine;
                # keeping it off scalar stops ACTIVATEs from stalling the Act
                # DMA ring's issue stream
                nc.vector.tensor_scalar_add(ot[:], ps[:], bt[:, p : p + 1])
                # final stores ride the HW rings, which have drained their
                # loads by then; earlier stores stay on the SWDGE ring
                if p == PL - 1:
                    nc.sync.dma_start(y[p, :, : B // 2], ot[:, : B // 2])
                    nc.scalar.dma_start(y[p, :, B // 2 :], ot[:, B // 2 :])
                elif p == PL - 2:
                    nc.scalar.dma_start(y[p], ot[:])
                else:
                    nc.gpsimd.dma_start(y[p], ot[:])
    nc.compile()
    return nc


def _shard_inputs(x: np.ndarray, W: np.ndarray, b: np.ndarray):
    # xk[p, k, c*B + b] = x[b, 128c + k, p]
    xk = (
        x.transpose(2, 1, 0)
        .reshape(P, KC, 128, B)
        .transpose(0, 2, 1, 3)
        .reshape(P, 128, KC * B)
    )
    # wk[p, k, c*E + e] = W[p, 128c + k, e]
    wk = W.reshape(P, KC, 128, E).transpose(0, 2, 1, 3).reshape(P, 128, KC * E)
    bT = b.T  # [E, P]
    in_maps = []
    for m in range(NCORES):
        sl = slice(m * PL, (m + 1) * PL)
        in_maps.append(
            {
                "xk": np.ascontiguousarray(xk[sl]),
                "wk": np.ascontiguousarray(wk[sl]),
                "bT": np.ascontiguousarray(bT[:, sl]),
            }
        )
    return in_maps


def kernel(x: np.ndarray, W: np.ndarray, b: np.ndarray) -> np.ndarray:
    global _nc_cache, LAST_RESULTS
    x = np.ascontiguousarray(np.asarray(x, dtype=np.float32))
    W = np.ascontiguousarray(np.asarray(W, dtype=np.float32))
    b = np.ascontiguousarray(np.asarray(b, dtype=np.float32))
    if _nc_cache is None:
        _nc_cache = _build_nc()
    in_maps = _shard_inputs(x, W, b)
    res = run_bass_kernel_spmd(
        _nc_cache,
        in_maps,
        core_ids=list(range(NCORES)),
        trace=bool(os.environ.get("KERNEL_TRACE")),
    )
    LAST_RESULTS = res
    yall = np.concatenate([r["y"] for r in res.results], axis=0)  # [P, E, B]
    return np.ascontiguousarray(yall.transpose(2, 1, 0))  # [B, E, P]


# revision 26
# speedup vs baseline: 1.0311x; 1.0311x over previous
"""Trainium2 Bass kernel for nn_Loop_Projection (batched per-prototype GEMM).

Computes out[b, e, p] = sum_d x[b, d, p] * W[p, d, e] + b[p, e] with
x: [256, 512, 128] f32, W: [128, 512, 128] f32, b: [128, 128] f32.

Sharding: prototype axis P=128 split across 8 NeuronCores (16 protos each).
Each core's x/W slices are pre-transposed on the host so every device DMA is
fully contiguous:
  xk[p][k, c*B + b] = x[b, 128c + k, p]      ([16, 128, 1024] per core)
  wk[p][k, c*E + e] = W[p, 128c + k, e]      ([16, 128, 512]  per core)
Per proto the kernel accumulates out.T = W_p.T @ x_p.T ([E, B] PSUM tile)
over 4 K-chunks of 128 (fp32 matmuls), adds the bias on the vector engine
during the PSUM->SBUF copy, and stores y[p] = [E, B] contiguous. The host
reassembles [B, E, P].

The device program is raw bacc (hand-placed semaphores, no Tile) so the
kernel has no Tile exit barrier. DMA traffic is spread over three rings --
x halves split across the two HWDGE rings (SP=sync + Act=scalar), W loads
alternating between them, stores on the gpsimd SWDGE ring -- which sustains
~400 GB/s aggregate vs ~260 GB/s for a single ring. Per-ring-slot DMA
semaphores are used because HWDGE completions of different DMAs can
interleave (only per-slot counts are race-free). Measured ~54-55 us on
8 cores (12 MiB in + 2 MiB out per core), rel err ~1e-7.
"""

import os

import numpy as np

import concourse.bass as bass
import concourse.tile as tile
from concourse import bacc, mybir
from concourse.bass_utils import run_bass_kernel_spmd

B, D, P, E = 256, 512, 128, 128
NCORES = 8
PL = P // NCORES  # prototypes per core
KC = D // 128  # contraction chunks of 128

_nc_cache = None
LAST_RESULTS = None  # BassKernelResults of the most recent run (for test.py)

USE_FP32R = False  # float32r matmul: 1 cycle/row vs 4 for float32
USE_RAW = True  # raw bacc (manual sems) instead of Tile: no ~9us exit barrier

NB = 12  # x/w sbuf ring depth
NPS = 8  # psum ring depth (8 banks)
NO = 16  # output slots: single-use, so no store-completion reuse guards


def _build_nc_raw() -> bass.Bass:
    nc = bacc.Bacc()
    xk = nc.dram_tensor("xk", [PL, 128, KC * B], mybir.dt.float32, kind="ExternalInput")
    wk = nc.dram_tensor("wk", [PL, 128, KC * E], mybir.dt.float32, kind="ExternalInput")
    bT = nc.dram_tensor("bT", [E, PL], mybir.dt.float32, kind="ExternalInput")
    y = nc.dram_tensor("y", [PL, E, B], mybir.dt.float32, kind="ExternalOutput")

    mm_dt = mybir.dt.float32r if USE_FP32R else mybir.dt.float32
    XW = KC * B  # 1024
    XH = XW // 2  # 512, per-ring half of an x tile

    # store issuer per proto: last two protos ride the HW rings (idle by then)
    def store_engine(p):
        if p == PL - 1:
            return "split"
        if p == PL - 2:
            return "act"
        return "pool"


    if True:
        # plain allocs (no context managers): freeing sems/tensors at the end
        # of the program emits a ~7us per-semaphore clear storm at kernel exit
        xbuf = [
            nc.alloc_sbuf_tensor(f"xbuf{i}", [128, XW], mm_dt).ap()
            for i in range(NB)
        ]
        wbuf = [
            nc.alloc_sbuf_tensor(f"wbuf{i}", [128, KC * E], mm_dt).ap()
            for i in range(NB)
        ]
        obuf = [
            nc.alloc_sbuf_tensor(f"obuf{i}", [E, B], mybir.dt.float32).ap()
            for i in range(NO)
        ]
        pbuf = [
            nc.alloc_psum_tensor(f"pbuf{i}", [E, B], mybir.dt.float32).ap()
            for i in range(NPS)
        ]
        btile = nc.alloc_sbuf_tensor("btile", [E, PL], mybir.dt.float32).ap()
        # one DMA-completion sem per ring slot: same-slot uses are serialized
        # by the ring guard, so per-slot counting is sound even though HWDGE
        # completions of different DMAs can interleave
        s_x = [nc.alloc_semaphore(f"s_x{i}") for i in range(NB)]
        s_st = nc.alloc_semaphore("s_st")
        s_st_hw = nc.alloc_semaphore("s_st_hw")
        s_b = nc.alloc_semaphore("s_b")
        s_mm = nc.alloc_semaphore("s_mm")
        s_vec = nc.alloc_semaphore("s_vec")

        with nc.Block() as block:

            @block.sync
            def _(sync: bass.BassEngine):
                for p in range(PL):
                    if p >= NB:
                        sync.wait_ge(s_mm, p - NB + 1)
                    sync.dma_start(
                        xbuf[p % NB][:, :XH], xk[p, :, :XH].bitcast(mm_dt)
                    ).then_inc(s_x[p % NB], 16)
                    if p % 2 == 0:
                        sync.dma_start(
                            wbuf[p % NB][:], wk[p].bitcast(mm_dt)
                        ).then_inc(s_x[p % NB], 16)
                p = PL - 2
                sync.wait_ge(s_vec, PL - 1)
                sync.dma_start(
                    y[p, :, : B // 2], obuf[p % NO][:, : B // 2]
                ).then_inc(s_st_hw, 16)
                p = PL - 1
                sync.wait_ge(s_vec, PL)
                sync.dma_start(
                    y[p, :, : B // 2], obuf[p % NO][:, : B // 2]
                ).then_inc(s_st_hw, 16)
                sync.wait_ge(s_st_hw, 64)

            @block.scalar
            def _(scalar: bass.BassEngine):
                for p in range(PL):
                    if p >= NB:
                        scalar.wait_ge(s_mm, p - NB + 1)
                    scalar.dma_start(
                        xbuf[p % NB][:, XH:], xk[p, :, XH:].bitcast(mm_dt)
                    ).then_inc(s_x[p % NB], 16)
                    if p % 2 == 1:
                        scalar.dma_start(
                            wbuf[p % NB][:], wk[p].bitcast(mm_dt)
                        ).then_inc(s_x[p % NB], 16)
                p = PL - 2
                scalar.wait_ge(s_vec, p + 1)
                scalar.dma_start(
                    y[p, :, B // 2 :], obuf[p % NO][:, B // 2 :]
                ).then_inc(s_st_hw, 16)
                p = PL - 1
                scalar.wait_ge(s_vec, p + 1)
                scalar.dma_start(
                    y[p, :, B // 2 :], obuf[p % NO][:, B // 2 :]
                ).then_inc(s_st_hw, 16)
                scalar.wait_ge(s_st_hw, 64)

            @block.tensor
            def _(tensor: bass.BassEngine):
                for p in range(PL):
                    i = p % NB
                    use = p // NB + 1
                    # both x halves + this slot's W load (3 DMAs x 16)
                    tensor.wait_ge(s_x[i], 48 * use)
                    if p >= NPS:
                        tensor.wait_ge(s_vec, p - NPS + 1)
                    for c in range(KC):
                        mm = nc.tensor.matmul(
                            pbuf[p % NPS][:],
                            lhsT=wbuf[i][:, c * E : (c + 1) * E],
                            rhs=xbuf[i][:, c * B : (c + 1) * B],
                            start=(c == 0),
                            stop=(c == KC - 1),
                        )
                    mm.then_inc(s_mm, 1)

            @block.vector
            def _(vector: bass.BassEngine):
                vector.wait_ge(s_b, 16)
                for p in range(PL):
                    vector.wait_ge(s_mm, p + 1)
                    nc.vector.tensor_scalar_add(
                        obuf[p % NO][:], pbuf[p % NPS][:], btile[:, p : p + 1]
                    ).then_inc(s_vec, 1)

            @block.gpsimd
            def _(gpsimd: bass.BassEngine):
                # bias rides the otherwise-idle SWDGE ring, off the Act ring head
                gpsimd.dma_start(btile[:], bT[:]).then_inc(s_b, 16)
                for p in range(PL):
                    if store_engine(p) != "pool":
                        continue
                    gpsimd.wait_ge(s_vec, p + 1)
                    gpsimd.dma_start(y[p], obuf[p % NO][:]).then_inc(s_st, 16)
                gpsimd.wait_ge(s_st, 16 * (PL - 2))

    nc.compile()
    return nc


def _build_nc() -> bass.Bass:
    if USE_RAW:
        return _build_nc_raw()
    nc = bacc.Bacc()
    xk = nc.dram_tensor("xk", [PL, 128, KC * B], mybir.dt.float32, kind="ExternalInput")
    wk = nc.dram_tensor("wk", [PL, 128, KC * E], mybir.dt.float32, kind="ExternalInput")
    bT = nc.dram_tensor("bT", [E, PL], mybir.dt.float32, kind="ExternalInput")
    y = nc.dram_tensor("y", [PL, E, B], mybir.dt.float32, kind="ExternalOutput")

    mm_dt = mybir.dt.float32r if USE_FP32R else mybir.dt.float32
    XW = KC * B  # 1024, x tile free width
    with tile.TileContext(nc) as tc:
        with (
            tc.tile_pool(name="const", bufs=1) as cpool,
            tc.tile_pool(name="xin", bufs=8) as xpool,
            tc.tile_pool(name="win", bufs=8) as wpool,
            tc.tile_pool(name="acc", bufs=8, space="PSUM") as ppool,
            tc.tile_pool(name="out", bufs=8) as opool,
        ):
            bt = cpool.tile([E, PL], mybir.dt.float32)
            nc.scalar.dma_start(bt[:], bT[:])
            for p in range(PL):
                # Split each x load across both HWDGE rings (SP + Act) and
                # alternate the W loads so both rings carry ~6 MiB; stores
                # ride the gpsimd SWDGE ring. One ring alone caps at ~260
                # GB/s, below the ~358 GB/s HBM-per-core limit.
                xt = xpool.tile([128, XW], mm_dt)
                nc.sync.dma_start(
                    xt[:, : XW // 2], xk[p, :, : XW // 2].bitcast(mm_dt)
                )
                nc.scalar.dma_start(
                    xt[:, XW // 2 :], xk[p, :, XW // 2 :].bitcast(mm_dt)
                )
                wt = wpool.tile([128, KC * E], mm_dt)
                weng = nc.sync if p % 2 == 0 else nc.scalar
                weng.dma_start(wt[:], wk[p].bitcast(mm_dt))
                ps = ppool.tile([E, B], mybir.dt.float32)
                for c in range(KC):
                    nc.tensor.matmul(
                        ps[:],
                        lhsT=wt[:, c * E : (c + 1) * E],
                        rhs=xt[:, c * B : (c + 1) * B],
                        start=(c == 0),
                        stop=(c == KC - 1),
                    )
                ot = opool.tile([E, B], mybir.dt.float32)
                # bias-add + PSUM->SBUF on the (otherwise idle) vector engine;
                # keeping it off scalar stops ACTIVATEs from stalling the Act
                # DMA ring's issue stream
                nc.vector.tensor_scalar_add(ot[:], ps[:], bt[:, p : p + 1])
                # final stores ride the HW rings, which have drained their
                # loads by then; earlier stores stay on the SWDGE ring
                if p == PL - 1:
                    nc.sync.dma_start(y[p, :, : B // 2], ot[:, : B // 2])
                    nc.scalar.dma_start(y[p, :, B // 2 :], ot[:, B // 2 :])
                elif p == PL - 2:
                    nc.scalar.dma_start(y[p], ot[:])
                else:
                    nc.gpsimd.dma_start(y[p], ot[:])
    nc.compile()
    return nc


def _shard_inputs(x: np.ndarray, W: np.ndarray, b: np.ndarray):
    # xk[p, k, c*B + b] = x[b, 128c + k, p]
    xk = (
        x.transpose(2, 1, 0)
        .reshape(P, KC, 128, B)
        .transpose(0, 2, 1, 3)
        .reshape(P, 128, KC * B)
    )
    # wk[p, k, c*E + e] = W[p, 128c + k, e]
    wk = W.reshape(P, KC, 128, E).transpose(0, 2, 1, 3).reshape(P, 128, KC * E)
    bT = b.T  # [E, P]
    in_maps = []
    for m in range(NCORES):
        sl = slice(m * PL, (m + 1) * PL)
        in_maps.append(
            {
                "xk": np.ascontiguousarray(xk[sl]),
                "wk": np.ascontiguousarray(wk[sl]),
                "bT": np.ascontiguousarray(bT[:, sl]),
            }
        )
    return in_maps


def kernel(x: np.ndarray, W: np.ndarray, b: np.ndarray) -> np.ndarray:
    global _nc_cache, LAST_RESULTS
    x = np.ascontiguousarray(np.asarray(x, dtype=np.float32))
    W = np.ascontiguousarray(np.asarray(W, dtype=np.float32))
    b = np.ascontiguousarray(np.asarray(b, dtype=np.float32))
    if _nc_cache is None:
        _nc_cache = _build_nc()
    in_maps = _shard_inputs(x, W, b)
    res = run_bass_kernel_spmd(
        _nc_cache,
        in_maps,
        core_ids=list(range(NCORES)),
        trace=bool(os.environ.get("KERNEL_TRACE")),
    )
    LAST_RESULTS = res
    yall = np.concatenate([r["y"] for r in res.results], axis=0)  # [P, E, B]
    return np.ascontiguousarray(yall.transpose(2, 1, 0))  # [B, E, P]
